# revision 9
# baseline (speedup 1.0000x reference)
"""Pairwise squared-euclidean distance kernel for Trainium2 (8 NeuronCores).

z[i, j] = ||x_i||^2 + ||y_j||^2 - 2 * <x_i, y_j>

Sharding: x rows split across 8 cores (1024 rows each), y replicated.
Each core computes a [1024, 8192] tile of the output with no communication.

Per-core algorithm (fp32 data; cross terms in fp16 on the PE):
  1. Load x shard, xsq row norms (ScalarE Square+accum), PE-transpose x
     casting to fp16 and folding the -2 scale during PSUM evacuation.
  2. Single n-outer pipeline over 16 chunks of 512 y rows:
     a. DMA chunk in (ScalarE HWDGE ring, decoupled from the out stream).
     b. PE-transpose (fp32) into yT fp16 (VectorE cast evac); ScalarE
        squares the same transpose PSUM into yTsq fp16.
     c. Per m-tile PSUM bank: accumulate ones128^T @ yTsq (adds ||y_j||^2
        broadcast to all partitions -- no elementwise ysq add anywhere)
        plus the two fp16 cross-term matmuls with stationary xT slices.
     d. Evacuate adding xsq as per-partition bias (ScalarE bias path /
        VectorE tensor_scalar_add alternating), DMA the [128, 512]
        stripe out (Sync HWDGE ring).
     The Tile scheduler pipelines chunks: output DMA starts after the
     first chunk instead of after all y preprocessing.

Known-good environment notes: tensor_tensor_reduce crashes the device
(NRT_EXEC_UNIT_UNRECOVERABLE) - do not use. fp32r matmuls never warm
the HAM clock gate and self-load weights serially (~536ns/mm).
GpSimd fp32 SBUF adds run at ~53 G elem/s (3x slower than docs claim)
- keep it off the critical path.
"""

import os

import numpy as np

import concourse.bacc as bacc
import concourse.mybir as mybir
import concourse.tile as tile
from concourse.bass_utils import run_bass_kernel_spmd
from concourse.masks import make_identity

N_CORES = 8
N_FULL = 8192  # total x rows
M_Y = 8192  # y rows
D = 256  # feature dim
N_SHARD = N_FULL // N_CORES  # 1024 x rows per core

P = 128
NT = 512  # matmul free-dim tile (one fp32 PSUM bank)
FP32 = mybir.dt.float32
FP16 = mybir.dt.float16
AF = mybir.ActivationFunctionType
ALU = mybir.AluOpType

_CACHE = {}
LAST_RESULTS = None


def _build():
    nc = bacc.Bacc("TRN2", target_bir_lowering=False, debug=False, num_devices=N_CORES)
    x_d = nc.dram_tensor("x", [N_SHARD, D], FP32, kind="ExternalInput").ap()
    y_d = nc.dram_tensor("y", [M_Y, D], FP32, kind="ExternalInput").ap()
    out_d = nc.dram_tensor("out", [N_SHARD, M_Y], FP32, kind="ExternalOutput").ap()

    M_TILES = N_SHARD // P  # 8 m-tiles (x rows)
    N_CHUNKS = M_Y // NT  # 16 chunks of 512 y rows / output cols

    with tile.TileContext(nc) as tc:
        with (
            tc.tile_pool(name="const", bufs=1) as const,
            tc.tile_pool(name="ystage", bufs=6) as ystage,
            tc.tile_pool(name="sq", bufs=4) as sqp,
            tc.tile_pool(name="outp", bufs=16) as outp,
            tc.tile_pool(name="psmm", bufs=8, space="PSUM") as psmm,
        ):
            identity = const.tile([P, P], FP32)
            make_identity(nc, identity)
            ones128 = const.tile([P, P], FP16)
            nc.gpsimd.memset(ones128[:], 1.0)

            xsq = const.tile([P, M_TILES], FP32)
            x_nat = const.tile([P, M_TILES, D], FP32)
            xT = [
                const.tile([P, N_SHARD], FP16, tag=f"xT{c}", name=f"xT{c}")
                for c in range(2)
            ]
            yT = [
                const.tile([P, M_Y], FP16, tag=f"yT{c}", name=f"yT{c}")
                for c in range(2)
            ]
            yTsq = [
                const.tile([P, M_Y], FP16, tag=f"yTsq{c}", name=f"yTsq{c}")
                for c in range(2)
            ]
            # elementwise sum of the two yTsq halves (GpSimd, otherwise idle):
            # one ones128^T @ yTs matmul then yields the full 256-dim ysq sum.
            yTs = const.tile([P, M_Y], FP16, tag="yTs", name="yTs")

            # ---- main pipeline: per 512-row / 512-col y chunk ----
            # The transpose stage runs two chunks ahead of the matmul stage
            # so the yT/yTsq/yTs chain (PE transpose -> ACT square -> GpSimd
            # halves-sum) completes a full iteration before its matmuls.
            def load_transpose_chunk(n):
                nsl = slice(n * NT, (n + 1) * NT)
                yst = ystage.tile([P, 4, D], FP32, tag="yst")
                nc.scalar.dma_start(
                    yst[:],
                    y_d[n * NT : (n + 1) * NT, :].rearrange(
                        "(t p) d -> p t d", p=P
                    ),
                )
                for c in range(2):
                    ps = psmm.tile([P, NT], FP32, tag="mm")
                    for s in range(4):
                        nc.tensor.transpose(
                            ps[:, s * P : (s + 1) * P],
                            yst[:, s, c * P : (c + 1) * P],
                            identity,
                        )
                    nc.vector.tensor_copy(yT[c][:, nsl], ps[:])
                    nc.scalar.activation(yTsq[c][:, nsl], ps[:], AF.Square)
                nc.gpsimd.tensor_tensor(
                    yTs[:, nsl], yTsq[0][:, nsl], yTsq[1][:, nsl], ALU.add
                )

            load_transpose_chunk(0)

            # ---- x: load, row norms, transpose (x -2 folded into evac) ----
            # Emitted after chunk 0 so the PE starts on y transposes ASAP;
            # the x DMA (sync ring) overlaps the y loads (scalar ring).
            nc.sync.dma_start(x_nat[:], x_d.rearrange("(t p) d -> p t d", p=P))
            for t in range(M_TILES):
                sq = sqp.tile([P, D], FP32, tag="sq")
                nc.scalar.activation(
                    sq[:], x_nat[:, t, :], AF.Square, accum_out=xsq[:, t : t + 1]
                )
            for c in range(2):
                for h in range(2):
                    ps = psmm.tile([P, NT], FP32, tag="mm")
                    for s in range(4):
                        t = h * 4 + s
                        nc.tensor.transpose(
                            ps[:, s * P : (s + 1) * P],
                            x_nat[:, t, c * P : (c + 1) * P],
                            identity,
                        )
                    nc.vector.tensor_scalar_mul(
                        xT[c][:, h * NT : (h + 1) * NT], ps[:], -2.0
                    )

            load_transpose_chunk(1)
            for n in range(N_CHUNKS):
                nsl = slice(n * NT, (n + 1) * NT)
                if n + 2 < N_CHUNKS:
                    load_transpose_chunk(n + 2)

                pms = [
                    psmm.tile([P, NT], FP32, tag="mm", name=f"pm_{n}_{m}")
                    for m in range(M_TILES)
                ]
                for m in range(M_TILES):
                    msl = slice(m * P, (m + 1) * P)
                    nc.tensor.matmul(
                        pms[m][:], ones128[:], yTs[:, nsl],
                        start=True, stop=False,
                    )
                    nc.tensor.matmul(
                        pms[m][:], xT[0][:, msl], yT[0][:, nsl],
                        start=False, stop=False,
                    )
                    nc.tensor.matmul(
                        pms[m][:], xT[1][:, msl], yT[1][:, nsl],
                        start=False, stop=True,
                    )
                for m in range(M_TILES):
                    ot = outp.tile([P, NT], FP32, tag="ot")
                    if m % 2 == 0:
                        nc.scalar.activation(
                            ot[:], pms[m][:], AF.Identity,
                            bias=xsq[:, m : m + 1], scale=1.0,
                        )
                    else:
                        nc.vector.tensor_scalar_add(
                            ot[:], pms[m][:], xsq[:, m : m + 1]
                        )
                    nc.sync.dma_start(
                        out_d[m * P : (m + 1) * P, n * NT : (n + 1) * NT], ot[:]
                    )

    nc.compile()
    return nc


def _get_nc():
    if "nc" not in _CACHE:
        _CACHE["nc"] = _build()
    return _CACHE["nc"]


def kernel(x: np.ndarray, y: np.ndarray) -> np.ndarray:
    global LAST_RESULTS
    x = np.ascontiguousarray(np.asarray(x, dtype=np.float32))
    y = np.ascontiguousarray(np.asarray(y, dtype=np.float32))
    assert x.shape == (N_FULL, D) and y.shape == (M_Y, D)

    nc = _get_nc()
    in_maps = [
        {"x": x[i * N_SHARD : (i + 1) * N_SHARD], "y": y} for i in range(N_CORES)
    ]
    res = run_bass_kernel_spmd(
        nc,
        in_maps,
        core_ids=list(range(N_CORES)),
        trace=bool(os.environ.get("BASS_KERNEL_TRACE")),
    )
    LAST_RESULTS = res
    return np.concatenate([res.results[i]["out"] for i in range(N_CORES)], axis=0)


# revision 13
# speedup vs baseline: 1.0164x; 1.0164x over previous
"""Pairwise squared-euclidean distance kernel for Trainium2 (8 NeuronCores).

z[i, j] = ||x_i||^2 + ||y_j||^2 - 2 * <x_i, y_j>

Sharding: x rows split across 8 cores (1024 rows each), y replicated.
Each core computes a [1024, 8192] tile of the output with no communication.

Per-core algorithm (fp32 data; cross terms in fp16 on the PE):
  1. Load x shard, xsq row norms (ScalarE Square+accum), PE-transpose x
     casting to fp16 and folding the -2 scale during PSUM evacuation.
  2. Single n-outer pipeline over 16 chunks of 512 y rows:
     a. DMA chunk in (ScalarE HWDGE ring, decoupled from the out stream).
     b. PE-transpose (fp32) into yT fp16 (VectorE cast evac); ScalarE
        squares the same transpose PSUM into yTsq fp16.
     c. Per m-tile PSUM bank: accumulate ones128^T @ yTsq (adds ||y_j||^2
        broadcast to all partitions -- no elementwise ysq add anywhere)
        plus the two fp16 cross-term matmuls with stationary xT slices.
     d. Evacuate adding xsq as per-partition bias (ScalarE bias path /
        VectorE tensor_scalar_add alternating), DMA the [128, 512]
        stripe out (Sync HWDGE ring).
     The Tile scheduler pipelines chunks: output DMA starts after the
     first chunk instead of after all y preprocessing.

Known-good environment notes: tensor_tensor_reduce crashes the device
(NRT_EXEC_UNIT_UNRECOVERABLE) - do not use. fp32r matmuls never warm
the HAM clock gate and self-load weights serially (~536ns/mm).
GpSimd fp32 SBUF adds run at ~53 G elem/s (3x slower than docs claim)
- keep it off the critical path.
"""

import os

import numpy as np

import concourse.bacc as bacc
import concourse.mybir as mybir
import concourse.tile as tile
from concourse.bass_utils import run_bass_kernel_spmd
from concourse.masks import make_identity

N_CORES = 8
N_FULL = 8192  # total x rows
M_Y = 8192  # y rows
D = 256  # feature dim
N_SHARD = N_FULL // N_CORES  # 1024 x rows per core

P = 128
NT = 512  # matmul free-dim tile (one fp32 PSUM bank)
FP32 = mybir.dt.float32
FP16 = mybir.dt.float16
AF = mybir.ActivationFunctionType
ALU = mybir.AluOpType

_CACHE = {}
LAST_RESULTS = None


def _build():
    nc = bacc.Bacc("TRN2", target_bir_lowering=False, debug=False, num_devices=N_CORES)
    x_d = nc.dram_tensor("x", [N_SHARD, D], FP32, kind="ExternalInput").ap()
    y_d = nc.dram_tensor("y", [M_Y, D], FP32, kind="ExternalInput").ap()
    out_d = nc.dram_tensor("out", [N_SHARD, M_Y], FP32, kind="ExternalOutput").ap()

    M_TILES = N_SHARD // P  # 8 m-tiles (x rows)
    N_CHUNKS = M_Y // NT  # 16 chunks of 512 y rows / output cols

    with tile.TileContext(nc) as tc:
        with (
            tc.tile_pool(name="const", bufs=1) as const,
            tc.tile_pool(name="ystage", bufs=6) as ystage,
            tc.tile_pool(name="sq", bufs=4) as sqp,
            tc.tile_pool(name="outp", bufs=24) as outp,
            tc.tile_pool(name="psmm", bufs=8, space="PSUM") as psmm,
        ):
            identity = const.tile([P, P], FP32)
            make_identity(nc, identity)
            ones128 = const.tile([P, P], FP16)
            nc.gpsimd.memset(ones128[:], 1.0)

            xsq = const.tile([P, M_TILES], FP32)
            x_nat = const.tile([P, M_TILES, D], FP32)
            xT = [
                const.tile([P, N_SHARD], FP16, tag=f"xT{c}", name=f"xT{c}")
                for c in range(2)
            ]
            yT = [
                const.tile([P, M_Y], FP16, tag=f"yT{c}", name=f"yT{c}")
                for c in range(2)
            ]
            yTsq = [
                const.tile([P, M_Y], FP16, tag=f"yTsq{c}", name=f"yTsq{c}")
                for c in range(2)
            ]
            # elementwise sum of the two yTsq halves (GpSimd, otherwise idle):
            # one ones128^T @ yTs matmul then yields the full 256-dim ysq sum.
            yTs = const.tile([P, M_Y], FP16, tag="yTs", name="yTs")

            # ---- x: load, row norms, transpose (x -2 folded into evac) ----
            # Emitted first so the x PSUM tiles are evacuated (and their
            # slots released) while the first y chunks are still loading.
            nc.sync.dma_start(x_nat[:], x_d.rearrange("(t p) d -> p t d", p=P))
            for t in range(M_TILES):
                sq = sqp.tile([P, D], FP32, tag="sq")
                nc.scalar.activation(
                    sq[:], x_nat[:, t, :], AF.Square, accum_out=xsq[:, t : t + 1]
                )
            for c in range(2):
                for h in range(2):
                    ps = psmm.tile([P, NT], FP32, tag="mm")
                    for s in range(4):
                        t = h * 4 + s
                        nc.tensor.transpose(
                            ps[:, s * P : (s + 1) * P],
                            x_nat[:, t, c * P : (c + 1) * P],
                            identity,
                        )
                    nc.vector.tensor_scalar_mul(
                        xT[c][:, h * NT : (h + 1) * NT], ps[:], -2.0
                    )

            # ---- main pipeline: per 512-row / 512-col y chunk ----
            # The transpose stage runs one chunk ahead of the matmul stage
            # so the PE fills its wait-for-evac gap with the next chunk's
            # transposes and the yT/yTsq evacs overlap the matmul block.
            def load_transpose_chunk(n):
                nsl = slice(n * NT, (n + 1) * NT)
                yst = ystage.tile([P, 4, D], FP32, tag="yst")
                nc.scalar.dma_start(
                    yst[:],
                    y_d[n * NT : (n + 1) * NT, :].rearrange(
                        "(t p) d -> p t d", p=P
                    ),
                )
                for c in range(2):
                    ps = psmm.tile([P, NT], FP32, tag="mm")
                    for s in range(4):
                        nc.tensor.transpose(
                            ps[:, s * P : (s + 1) * P],
                            yst[:, s, c * P : (c + 1) * P],
                            identity,
                        )
                    nc.vector.tensor_copy(yT[c][:, nsl], ps[:])
                    nc.scalar.activation(yTsq[c][:, nsl], ps[:], AF.Square)
                nc.gpsimd.tensor_tensor(
                    yTs[:, nsl], yTsq[0][:, nsl], yTsq[1][:, nsl], ALU.add
                )

            load_transpose_chunk(0)
            for n in range(N_CHUNKS):
                nsl = slice(n * NT, (n + 1) * NT)
                if n + 1 < N_CHUNKS:
                    load_transpose_chunk(n + 1)

                pms = [
                    psmm.tile([P, NT], FP32, tag="mm", name=f"pm_{n}_{m}")
                    for m in range(M_TILES)
                ]
                # ysq matmul last: the GpSimd-produced yTs is the latest
                # input in the per-chunk dependency chain, so banks start
                # on the cross terms (which only need yT) first.
                for m in range(M_TILES):
                    msl = slice(m * P, (m + 1) * P)
                    nc.tensor.matmul(
                        pms[m][:], xT[0][:, msl], yT[0][:, nsl],
                        start=True, stop=False,
                    )
                    nc.tensor.matmul(
                        pms[m][:], xT[1][:, msl], yT[1][:, nsl],
                        start=False, stop=False,
                    )
                    nc.tensor.matmul(
                        pms[m][:], ones128[:], yTs[:, nsl],
                        start=False, stop=True,
                    )
                for m in range(M_TILES):
                    ot = outp.tile([P, NT], FP32, tag="ot")
                    if m % 2 == 0:
                        nc.scalar.activation(
                            ot[:], pms[m][:], AF.Identity,
                            bias=xsq[:, m : m + 1], scale=1.0,
                        )
                    else:
                        nc.vector.tensor_scalar_add(
                            ot[:], pms[m][:], xsq[:, m : m + 1]
                        )
                    nc.sync.dma_start(
                        out_d[m * P : (m + 1) * P, n * NT : (n + 1) * NT], ot[:]
                    )

    nc.compile()
    return nc


def _get_nc():
    if "nc" not in _CACHE:
        _CACHE["nc"] = _build()
    return _CACHE["nc"]


def kernel(x: np.ndarray, y: np.ndarray) -> np.ndarray:
    global LAST_RESULTS
    x = np.ascontiguousarray(np.asarray(x, dtype=np.float32))
    y = np.ascontiguousarray(np.asarray(y, dtype=np.float32))
    assert x.shape == (N_FULL, D) and y.shape == (M_Y, D)

    nc = _get_nc()
    in_maps = [
        {"x": x[i * N_SHARD : (i + 1) * N_SHARD], "y": y} for i in range(N_CORES)
    ]
    res = run_bass_kernel_spmd(
        nc,
        in_maps,
        core_ids=list(range(N_CORES)),
        trace=bool(os.environ.get("BASS_KERNEL_TRACE")),
    )
    LAST_RESULTS = res
    return np.concatenate([res.results[i]["out"] for i in range(N_CORES)], axis=0)


# revision 15
# speedup vs baseline: 1.0200x; 1.0035x over previous
"""Pairwise squared-euclidean distance kernel for Trainium2 (8 NeuronCores).

z[i, j] = ||x_i||^2 + ||y_j||^2 - 2 * <x_i, y_j>

Sharding: x rows split across 8 cores (1024 rows each), y replicated.
Each core computes a [1024, 8192] tile of the output with no communication.

Per-core algorithm (fp32 data; cross terms in fp16 on the PE):
  1. Load x shard, xsq row norms (ScalarE Square+accum), PE-transpose x
     casting to fp16 and folding the -2 scale during PSUM evacuation.
  2. Single n-outer pipeline over 16 chunks of 512 y rows:
     a. DMA chunk in (ScalarE HWDGE ring, decoupled from the out stream).
     b. PE-transpose (fp32) into yT fp16 (VectorE cast evac); ScalarE
        squares the same transpose PSUM into yTsq fp16.
     c. Per m-tile PSUM bank: accumulate ones128^T @ yTsq (adds ||y_j||^2
        broadcast to all partitions -- no elementwise ysq add anywhere)
        plus the two fp16 cross-term matmuls with stationary xT slices.
     d. Evacuate adding xsq as per-partition bias (ScalarE bias path /
        VectorE tensor_scalar_add alternating), DMA the [128, 512]
        stripe out (Sync HWDGE ring).
     The Tile scheduler pipelines chunks: output DMA starts after the
     first chunk instead of after all y preprocessing.

Known-good environment notes: tensor_tensor_reduce crashes the device
(NRT_EXEC_UNIT_UNRECOVERABLE) - do not use. fp32r matmuls never warm
the HAM clock gate and self-load weights serially (~536ns/mm).
GpSimd fp32 SBUF adds run at ~53 G elem/s (3x slower than docs claim)
- keep it off the critical path.
"""

import os

import numpy as np

import concourse.bacc as bacc
import concourse.mybir as mybir
import concourse.tile as tile
from concourse.bass_utils import run_bass_kernel_spmd
from concourse.masks import make_identity

N_CORES = 8
N_FULL = 8192  # total x rows
M_Y = 8192  # y rows
D = 256  # feature dim
N_SHARD = N_FULL // N_CORES  # 1024 x rows per core

P = 128
NT = 512  # matmul free-dim tile (one fp32 PSUM bank)
FP32 = mybir.dt.float32
FP16 = mybir.dt.float16
AF = mybir.ActivationFunctionType
ALU = mybir.AluOpType

_CACHE = {}
LAST_RESULTS = None


def _build():
    nc = bacc.Bacc("TRN2", target_bir_lowering=False, debug=False, num_devices=N_CORES)
    x_d = nc.dram_tensor("x", [N_SHARD, D], FP32, kind="ExternalInput").ap()
    y_d = nc.dram_tensor("y", [M_Y, D], FP32, kind="ExternalInput").ap()
    out_d = nc.dram_tensor("out", [N_SHARD, M_Y], FP32, kind="ExternalOutput").ap()

    M_TILES = N_SHARD // P  # 8 m-tiles (x rows)
    N_CHUNKS = M_Y // NT  # 16 chunks of 512 y rows / output cols

    with tile.TileContext(nc) as tc:
        with (
            tc.tile_pool(name="const", bufs=1) as const,
            tc.tile_pool(name="ystage", bufs=6) as ystage,
            tc.tile_pool(name="sq", bufs=4) as sqp,
            tc.tile_pool(name="outp", bufs=3) as outp,
            tc.tile_pool(name="psmm", bufs=8, space="PSUM") as psmm,
        ):
            identity = const.tile([P, P], FP32)
            make_identity(nc, identity)
            ones128 = const.tile([P, P], FP16)
            nc.gpsimd.memset(ones128[:], 1.0)

            xsq = const.tile([P, M_TILES], FP32)
            x_nat = const.tile([P, M_TILES, D], FP32)
            xT = [
                const.tile([P, N_SHARD], FP16, tag=f"xT{c}", name=f"xT{c}")
                for c in range(2)
            ]
            yT = [
                const.tile([P, M_Y], FP16, tag=f"yT{c}", name=f"yT{c}")
                for c in range(2)
            ]
            yTsq = [
                const.tile([P, M_Y], FP16, tag=f"yTsq{c}", name=f"yTsq{c}")
                for c in range(2)
            ]
            # elementwise sum of the two yTsq halves (GpSimd, otherwise idle):
            # one ones128^T @ yTs matmul then yields the full 256-dim ysq sum.
            yTs = const.tile([P, M_Y], FP16, tag="yTs", name="yTs")

            # ---- x: load, row norms, transpose (x -2 folded into evac) ----
            # Emitted first so the x PSUM tiles are evacuated (and their
            # slots released) while the first y chunks are still loading.
            nc.sync.dma_start(x_nat[:], x_d.rearrange("(t p) d -> p t d", p=P))
            for t in range(M_TILES):
                sq = sqp.tile([P, D], FP32, tag="sq")
                nc.scalar.activation(
                    sq[:], x_nat[:, t, :], AF.Square, accum_out=xsq[:, t : t + 1]
                )
            for c in range(2):
                for h in range(2):
                    ps = psmm.tile([P, NT], FP32, tag="mm")
                    for s in range(4):
                        t = h * 4 + s
                        nc.tensor.transpose(
                            ps[:, s * P : (s + 1) * P],
                            x_nat[:, t, c * P : (c + 1) * P],
                            identity,
                        )
                    nc.vector.tensor_scalar_mul(
                        xT[c][:, h * NT : (h + 1) * NT], ps[:], -2.0
                    )

            # ---- main pipeline: per 512-row / 512-col y chunk ----
            # The transpose stage runs one chunk ahead of the matmul stage
            # so the PE fills its wait-for-evac gap with the next chunk's
            # transposes and the yT/yTsq evacs overlap the matmul block.
            def load_transpose_chunk(n):
                nsl = slice(n * NT, (n + 1) * NT)
                yst = ystage.tile([P, 4, D], FP32, tag="yst")
                nc.scalar.dma_start(
                    yst[:],
                    y_d[n * NT : (n + 1) * NT, :].rearrange(
                        "(t p) d -> p t d", p=P
                    ),
                )
                for c in range(2):
                    ps = psmm.tile([P, NT], FP32, tag="mm")
                    for s in range(4):
                        nc.tensor.transpose(
                            ps[:, s * P : (s + 1) * P],
                            yst[:, s, c * P : (c + 1) * P],
                            identity,
                        )
                    nc.vector.tensor_copy(yT[c][:, nsl], ps[:])
                    nc.scalar.activation(yTsq[c][:, nsl], ps[:], AF.Square)
                nc.gpsimd.tensor_tensor(
                    yTs[:, nsl], yTsq[0][:, nsl], yTsq[1][:, nsl], ALU.add
                )

            load_transpose_chunk(0)
            for n in range(N_CHUNKS):
                nsl = slice(n * NT, (n + 1) * NT)
                if n + 1 < N_CHUNKS:
                    load_transpose_chunk(n + 1)

                pms = [
                    psmm.tile([P, NT], FP32, tag="mm", name=f"pm_{n}_{m}")
                    for m in range(M_TILES)
                ]
                # ysq matmul last: the GpSimd-produced yTs is the latest
                # input in the per-chunk dependency chain, so banks start
                # on the cross terms (which only need yT) first.
                for m in range(M_TILES):
                    msl = slice(m * P, (m + 1) * P)
                    nc.tensor.matmul(
                        pms[m][:], xT[0][:, msl], yT[0][:, nsl],
                        start=True, stop=False,
                    )
                    nc.tensor.matmul(
                        pms[m][:], xT[1][:, msl], yT[1][:, nsl],
                        start=False, stop=False,
                    )
                    nc.tensor.matmul(
                        pms[m][:], ones128[:], yTs[:, nsl],
                        start=False, stop=True,
                    )
                # one [128, 8, 512] tile per chunk -> a single 2 MB out DMA:
                # each dma_start costs the issuing sequencer ~700 ns
                # (DIRECT2D), so 8 stripes/chunk would eat most of the Sync
                # sequencer; merged, it is one issue per chunk.
                ot = outp.tile([P, M_TILES, NT], FP32, tag="ot")
                for m in range(M_TILES):
                    if m % 2 == 0:
                        nc.scalar.activation(
                            ot[:, m, :], pms[m][:], AF.Identity,
                            bias=xsq[:, m : m + 1], scale=1.0,
                        )
                    else:
                        nc.vector.tensor_scalar_add(
                            ot[:, m, :], pms[m][:], xsq[:, m : m + 1]
                        )
                nc.sync.dma_start(
                    out_d[:, n * NT : (n + 1) * NT].rearrange(
                        "(m p) j -> p m j", p=P
                    ),
                    ot[:],
                )

    nc.compile()
    return nc


def _get_nc():
    if "nc" not in _CACHE:
        _CACHE["nc"] = _build()
    return _CACHE["nc"]


def kernel(x: np.ndarray, y: np.ndarray) -> np.ndarray:
    global LAST_RESULTS
    x = np.ascontiguousarray(np.asarray(x, dtype=np.float32))
    y = np.ascontiguousarray(np.asarray(y, dtype=np.float32))
    assert x.shape == (N_FULL, D) and y.shape == (M_Y, D)

    nc = _get_nc()
    in_maps = [
        {"x": x[i * N_SHARD : (i + 1) * N_SHARD], "y": y} for i in range(N_CORES)
    ]
    res = run_bass_kernel_spmd(
        nc,
        in_maps,
        core_ids=list(range(N_CORES)),
        trace=bool(os.environ.get("BASS_KERNEL_TRACE")),
    )
    LAST_RESULTS = res
    return np.concatenate([res.results[i]["out"] for i in range(N_CORES)], axis=0)


# revision 16
# speedup vs baseline: 1.1371x; 1.1149x over previous
"""Pairwise squared-euclidean distance kernel for Trainium2 (8 NeuronCores).

z[i, j] = ||x_i||^2 + ||y_j||^2 - 2 * <x_i, y_j>

Sharding: x rows split across 8 cores (1024 rows each), y replicated.
Each core computes a [1024, 8192] tile of the output with no communication.

Per-core algorithm (fp32 data; cross terms in fp16 on the PE):
  1. Load x shard, xsq row norms (ScalarE Square+accum), PE-transpose x
     casting to fp16 and folding the -2 scale during PSUM evacuation.
  2. Single n-outer pipeline over 16 chunks of 512 y rows:
     a. DMA chunk in (ScalarE HWDGE ring, decoupled from the out stream).
     b. PE-transpose (fp32) into yT fp16 (VectorE cast evac); ScalarE
        squares the same transpose PSUM into yTsq fp16.
     c. Per m-tile PSUM bank: accumulate ones128^T @ yTsq (adds ||y_j||^2
        broadcast to all partitions -- no elementwise ysq add anywhere)
        plus the two fp16 cross-term matmuls with stationary xT slices.
     d. Evacuate adding xsq as per-partition bias (ScalarE bias path /
        VectorE tensor_scalar_add alternating), DMA the [128, 512]
        stripe out (Sync HWDGE ring).
     The Tile scheduler pipelines chunks: output DMA starts after the
     first chunk instead of after all y preprocessing.

Known-good environment notes: tensor_tensor_reduce crashes the device
(NRT_EXEC_UNIT_UNRECOVERABLE) - do not use. fp32r matmuls never warm
the HAM clock gate and self-load weights serially (~536ns/mm).
GpSimd fp32 SBUF adds run at ~53 G elem/s (3x slower than docs claim)
- keep it off the critical path.
"""

import os

import numpy as np

import concourse.bacc as bacc
import concourse.mybir as mybir
import concourse.tile as tile
from concourse.bass_utils import run_bass_kernel_spmd
from concourse.masks import make_identity

N_CORES = 8
N_FULL = 8192  # total x rows
M_Y = 8192  # y rows
D = 256  # feature dim
N_SHARD = N_FULL // N_CORES  # 1024 x rows per core

P = 128
NT = 512  # matmul free-dim tile (one fp32 PSUM bank)
FP32 = mybir.dt.float32
FP16 = mybir.dt.float16
AF = mybir.ActivationFunctionType
ALU = mybir.AluOpType

_CACHE = {}
LAST_RESULTS = None


def _build():
    nc = bacc.Bacc("TRN2", target_bir_lowering=False, debug=False, num_devices=N_CORES)
    x_d = nc.dram_tensor("x", [N_SHARD, D], FP32, kind="ExternalInput").ap()
    y_d = nc.dram_tensor("y", [M_Y, D], FP32, kind="ExternalInput").ap()
    out_d = nc.dram_tensor("out", [N_SHARD, M_Y], FP32, kind="ExternalOutput").ap()

    M_TILES = N_SHARD // P  # 8 m-tiles (x rows)
    N_CHUNKS = M_Y // NT  # 16 chunks of 512 y rows / output cols

    with tile.TileContext(nc) as tc:
        with (
            tc.tile_pool(name="const", bufs=1) as const,
            tc.tile_pool(name="ystage", bufs=10) as ystage,
            tc.tile_pool(name="sq", bufs=4) as sqp,
            tc.tile_pool(name="outp", bufs=3) as outp,
            tc.tile_pool(name="psmm", bufs=8, space="PSUM") as psmm,
        ):
            identity = const.tile([P, P], FP32)
            make_identity(nc, identity)
            ones128 = const.tile([P, P], FP16)
            nc.gpsimd.memset(ones128[:], 1.0)

            xsq = const.tile([P, M_TILES], FP32)
            x_nat = const.tile([P, M_TILES, D], FP32)
            xT = [
                const.tile([P, N_SHARD], FP16, tag=f"xT{c}", name=f"xT{c}")
                for c in range(2)
            ]
            yT = [
                const.tile([P, M_Y], FP16, tag=f"yT{c}", name=f"yT{c}")
                for c in range(2)
            ]
            yTsq = [
                const.tile([P, M_Y], FP16, tag=f"yTsq{c}", name=f"yTsq{c}")
                for c in range(2)
            ]
            # elementwise sum of the two yTsq halves (GpSimd, otherwise idle):
            # one ones128^T @ yTs matmul then yields the full 256-dim ysq sum.
            yTs = const.tile([P, M_Y], FP16, tag="yTs", name="yTs")

            # ---- x: load, row norms, transpose (x -2 folded into evac) ----
            # Emitted first so the x PSUM tiles are evacuated (and their
            # slots released) while the first y chunks are still loading.
            nc.sync.dma_start(x_nat[:], x_d.rearrange("(t p) d -> p t d", p=P))
            for t in range(M_TILES):
                sq = sqp.tile([P, D], FP32, tag="sq")
                nc.scalar.activation(
                    sq[:], x_nat[:, t, :], AF.Square, accum_out=xsq[:, t : t + 1]
                )
            for c in range(2):
                for h in range(2):
                    ps = psmm.tile([P, NT], FP32, tag="mm")
                    for s in range(4):
                        t = h * 4 + s
                        nc.tensor.transpose(
                            ps[:, s * P : (s + 1) * P],
                            x_nat[:, t, c * P : (c + 1) * P],
                            identity,
                        )
                    nc.vector.tensor_scalar_mul(
                        xT[c][:, h * NT : (h + 1) * NT], ps[:], -2.0
                    )

            # ---- main pipeline: per 512-row / 512-col y chunk ----
            # The transpose stage runs one chunk ahead of the matmul stage
            # so the PE fills its wait-for-evac gap with the next chunk's
            # transposes and the yT/yTsq evacs overlap the matmul block.
            def load_transpose_chunk(n):
                nsl = slice(n * NT, (n + 1) * NT)
                yst = ystage.tile([P, 4, D], FP32, tag="yst")
                nc.scalar.dma_start(
                    yst[:],
                    y_d[n * NT : (n + 1) * NT, :].rearrange(
                        "(t p) d -> p t d", p=P
                    ),
                )
                for c in range(2):
                    ps = psmm.tile([P, NT], FP32, tag="mm")
                    for s in range(4):
                        nc.tensor.transpose(
                            ps[:, s * P : (s + 1) * P],
                            yst[:, s, c * P : (c + 1) * P],
                            identity,
                        )
                    nc.vector.tensor_copy(yT[c][:, nsl], ps[:])
                    nc.scalar.activation(yTsq[c][:, nsl], ps[:], AF.Square)
                nc.gpsimd.tensor_tensor(
                    yTs[:, nsl], yTsq[0][:, nsl], yTsq[1][:, nsl], ALU.add
                )

            load_transpose_chunk(0)
            for n in range(N_CHUNKS):
                nsl = slice(n * NT, (n + 1) * NT)
                if n + 1 < N_CHUNKS:
                    load_transpose_chunk(n + 1)

                pms = [
                    psmm.tile([P, NT], FP32, tag="mm", name=f"pm_{n}_{m}")
                    for m in range(M_TILES)
                ]
                # ysq matmul last: the GpSimd-produced yTs is the latest
                # input in the per-chunk dependency chain, so banks start
                # on the cross terms (which only need yT) first.
                for m in range(M_TILES):
                    msl = slice(m * P, (m + 1) * P)
                    nc.tensor.matmul(
                        pms[m][:], xT[0][:, msl], yT[0][:, nsl],
                        start=True, stop=False,
                    )
                    nc.tensor.matmul(
                        pms[m][:], xT[1][:, msl], yT[1][:, nsl],
                        start=False, stop=False,
                    )
                    nc.tensor.matmul(
                        pms[m][:], ones128[:], yTs[:, nsl],
                        start=False, stop=True,
                    )
                # one [128, 8, 512] tile per chunk -> a single 2 MB out DMA:
                # each dma_start costs the issuing sequencer ~700 ns
                # (DIRECT2D), so 8 stripes/chunk would eat most of the Sync
                # sequencer; merged, it is one issue per chunk.
                ot = outp.tile([P, M_TILES, NT], FP32, tag="ot")
                for m in range(M_TILES):
                    if m % 2 == 0:
                        nc.scalar.activation(
                            ot[:, m, :], pms[m][:], AF.Identity,
                            bias=xsq[:, m : m + 1], scale=1.0,
                        )
                    else:
                        nc.vector.tensor_scalar_add(
                            ot[:, m, :], pms[m][:], xsq[:, m : m + 1]
                        )
                nc.sync.dma_start(
                    out_d[:, n * NT : (n + 1) * NT].rearrange(
                        "(m p) j -> p m j", p=P
                    ),
                    ot[:],
                )

    nc.compile()
    return nc


def _get_nc():
    if "nc" not in _CACHE:
        _CACHE["nc"] = _build()
    return _CACHE["nc"]


def kernel(x: np.ndarray, y: np.ndarray) -> np.ndarray:
    global LAST_RESULTS
    x = np.ascontiguousarray(np.asarray(x, dtype=np.float32))
    y = np.ascontiguousarray(np.asarray(y, dtype=np.float32))
    assert x.shape == (N_FULL, D) and y.shape == (M_Y, D)

    nc = _get_nc()
    in_maps = [
        {"x": x[i * N_SHARD : (i + 1) * N_SHARD], "y": y} for i in range(N_CORES)
    ]
    res = run_bass_kernel_spmd(
        nc,
        in_maps,
        core_ids=list(range(N_CORES)),
        trace=bool(os.environ.get("BASS_KERNEL_TRACE")),
    )
    LAST_RESULTS = res
    return np.concatenate([res.results[i]["out"] for i in range(N_CORES)], axis=0)
